# revision 2
# baseline (speedup 1.0000x reference)
"""AttentionContext TRN2 Bass kernel — data-parallel over batch on 8 NeuronCores.

Math (per sample b):
  query = Ws @ dec[b] + bs                       (K,)
  qh    = Wh.T @ query                           (H,)  [bh dropped: it only adds
          query.bh uniformly over t, which cancels in softmax]
  energy[t] = listener[b,t,:] . qh               (T,)  == query . keys[b,t,:] - query.bh
  e = exp(energy); Z = sum_t(e); s2 = sum_t(e*mask)
  att = e*mask / max(s2, EPS*Z)   == softmax -> mask -> L1-renorm of the reference
  ctxh = att @ listener[b]                       (H,)
  context = Wv @ ctxh + bv * sum(att)            (V,)

The big (B,T,K) keys and (B,T,V) value tensors are never materialized; the
kernel reads listener_output exactly once (memory-bound). On-chip layout:
listener tiles [t=128, h=512]; PE contracts t for ctxh (lhsT=att column), the
DVE fused multiply+reduce contracts h for energy. Energy/attention live as
[128, 16] per sample (col i = t-block i, partition p = t mod 128).
"""

from contextlib import ExitStack

import numpy as np

FP = None  # set on first build (mybir import deferred)

B = 64
B_LOC = 8      # samples per core
N_CORES = 8
T = 2048
H = 512
S = 512
K = 128
V = 128
NT = T // 128  # 16 t-blocks per sample
TG = 4         # t-blocks per DMA instruction (1 MiB)
EPS = 1e-12

_CACHE = {}


def _build(lpool_bufs: int = 12):
    import concourse.bacc as bacc
    import concourse.tile as tile
    import concourse.mybir as mybir

    FP = mybir.dt.float32
    AOP = mybir.AluOpType
    AFT = mybir.ActivationFunctionType

    nc = bacc.Bacc("TRN2", target_bir_lowering=False, debug=False,
                   enable_asserts=False)

    lst = nc.dram_tensor("lst", (B_LOC, T, H), FP, kind="ExternalInput").ap()
    decT = nc.dram_tensor("decT", (S, B_LOC), FP, kind="ExternalInput").ap()
    lens = nc.dram_tensor("lens", (1, B_LOC), FP, kind="ExternalInput").ap()
    wsT = nc.dram_tensor("wsT", (S, K), FP, kind="ExternalInput").ap()
    wh = nc.dram_tensor("wh", (K, H), FP, kind="ExternalInput").ap()
    wvT = nc.dram_tensor("wvT", (H, V), FP, kind="ExternalInput").ap()
    bs2 = nc.dram_tensor("bs2", (K, 1), FP, kind="ExternalInput").ap()
    bv2 = nc.dram_tensor("bv2", (1, V), FP, kind="ExternalInput").ap()
    iota = nc.dram_tensor("iota", (128, NT), FP, kind="ExternalInput").ap()
    ident = nc.dram_tensor("ident", (128, 128), FP, kind="ExternalInput").ap()
    ones = nc.dram_tensor("ones", (128, 128), FP, kind="ExternalInput").ap()

    ctx_out = nc.dram_tensor("ctx_out", (B_LOC, V), FP, kind="ExternalOutput").ap()
    att_out = nc.dram_tensor("att_out", (B_LOC, T), FP, kind="ExternalOutput").ap()

    with tile.TileContext(nc) as tc, ExitStack() as ctx:
        consts = ctx.enter_context(tc.tile_pool(name="consts", bufs=1))

        wsT_sb = consts.tile([128, 4, K], FP)
        nc.gpsimd.dma_start(out=wsT_sb, in_=wsT.rearrange("(j p) k -> p j k", p=128))
        wh_sb = consts.tile([K, H], FP)
        nc.gpsimd.dma_start(out=wh_sb, in_=wh)
        wvT_sb = consts.tile([128, 4, V], FP)
        nc.gpsimd.dma_start(out=wvT_sb, in_=wvT.rearrange("(j p) v -> p j v", p=128))
        decT_sb = consts.tile([128, 4, B_LOC], FP)
        nc.gpsimd.dma_start(out=decT_sb, in_=decT.rearrange("(j p) b -> p j b", p=128))
        bs_sb = consts.tile([K, 1], FP)
        nc.gpsimd.dma_start(out=bs_sb, in_=bs2)
        bv_sb = consts.tile([1, V], FP)
        nc.gpsimd.dma_start(out=bv_sb, in_=bv2)
        iota_sb = consts.tile([128, NT], FP)
        nc.gpsimd.dma_start(out=iota_sb, in_=iota)
        ident_sb = consts.tile([128, 128], FP)
        nc.gpsimd.dma_start(out=ident_sb, in_=ident)
        ones_sb = consts.tile([128, 128], FP)
        nc.gpsimd.dma_start(out=ones_sb, in_=ones)
        lens_sb = consts.tile([1, B_LOC], FP)
        nc.gpsimd.dma_start(out=lens_sb, in_=lens)

        # ---- front: queries, qh broadcast, masks for all samples ----
        qT_sb = consts.tile([K, B_LOC], FP)
        qhb_all = consts.tile([128, B_LOC, H], FP)
        lenb_sb = consts.tile([128, B_LOC], FP)
        mask_all = consts.tile([128, B_LOC, NT], FP)

        with tc.tile_pool(name="pfront", bufs=2, space="PSUM") as pfront:
            psum_qT = pfront.tile([K, B_LOC], FP, tag="pf")
            for j in range(4):
                nc.tensor.matmul(psum_qT, lhsT=wsT_sb[:, j, :], rhs=decT_sb[:, j, :],
                                 start=(j == 0), stop=(j == 3))
            nc.vector.tensor_scalar(out=qT_sb, in0=psum_qT, scalar1=bs_sb,
                                    scalar2=None, op0=AOP.add)
            psum_lenb = pfront.tile([128, B_LOC], FP, tag="pf")
            nc.tensor.matmul(psum_lenb, lhsT=ones_sb[0:1, :], rhs=lens_sb,
                             start=True, stop=True)
            nc.scalar.activation(out=lenb_sb, in_=psum_lenb, func=AFT.Copy)
            for b in range(B_LOC):
                nc.vector.tensor_scalar(out=mask_all[:, b, :], in0=iota_sb,
                                        scalar1=lenb_sb[:, b:b + 1], scalar2=None,
                                        op0=AOP.is_lt)
                psum_qh = pfront.tile([1, H], FP, tag="pqh")
                nc.tensor.matmul(psum_qh, lhsT=qT_sb[:, b:b + 1], rhs=wh_sb,
                                 start=True, stop=True)
                qh_sb = consts.tile([1, H], FP, tag="qh_sb")
                nc.scalar.activation(out=qh_sb, in_=psum_qh, func=AFT.Copy)
                psum_qhb = pfront.tile([128, H], FP, tag="pqh")
                nc.tensor.matmul(psum_qhb, lhsT=ones_sb[0:1, :], rhs=qh_sb,
                                 start=True, stop=True)
                nc.scalar.activation(out=qhb_all[:, b, :], in_=psum_qhb, func=AFT.Copy)

        # ---- main per-sample loop ----
        lpool = ctx.enter_context(tc.tile_pool(name="lpool", bufs=lpool_bufs))
        spool = ctx.enter_context(tc.tile_pool(name="spool", bufs=2))
        work = ctx.enter_context(tc.tile_pool(name="work", bufs=2))
        pctxh = ctx.enter_context(tc.tile_pool(name="pctxh", bufs=2, space="PSUM"))
        psml = ctx.enter_context(tc.tile_pool(name="psml", bufs=3, space="PSUM"))

        for b in range(B_LOC):
            lst_b = lst[b].rearrange("(g p) h -> p g h", p=128)
            lts = []
            for g in range(NT // TG):
                lt = lpool.tile([128, TG, H], FP, tag="lt")
                nc.sync.dma_start(out=lt, in_=lst_b[:, g * TG:(g + 1) * TG, :])
                lts.append(lt)

            # energy[p, i] = listener[b, 128*i+p, :] . qh
            energy = work.tile([128, NT], FP)
            for i in range(NT):
                scr = spool.tile([128, H], FP, tag="scr")
                nc.vector.affine_mul_reduce(
                    out=scr, accum_out=energy[:, i:i + 1],
                    in0=lts[i // TG][:, i % TG, :], in1=qhb_all[:, b, :],
                    scale=1.0, bias=0.0)

            # exp (+ row sums), masked exp (+ row sums)
            zs2 = work.tile([128, 2], FP)
            expe = work.tile([128, NT], FP)
            nc.scalar.activation(out=expe, in_=energy, func=AFT.Exp,
                                 accum_out=zs2[:, 0:1])
            masked = work.tile([128, NT], FP)
            nc.vector.affine_mul_reduce(
                out=masked, accum_out=zs2[:, 1:2], in0=expe,
                in1=mask_all[:, b, :], scale=1.0, bias=0.0)

            # cross-partition sums: psum_s[0,:] = [Z, s2]
            psum_s = psml.tile([1, 2], FP, tag="ps")
            nc.tensor.matmul(psum_s, lhsT=ones_sb[:, 0:1], rhs=zs2,
                             start=True, stop=True)
            # sml cols: 0=denom, 1=recip, 2=asum
            sml = work.tile([1, 3], FP)
            nc.vector.tensor_scalar(out=sml[0:1, 0:1], in0=psum_s[0:1, 0:1],
                                    scalar1=EPS, scalar2=None, op0=AOP.mult)
            nc.vector.tensor_tensor(out=sml[0:1, 0:1], in0=sml[0:1, 0:1],
                                    in1=psum_s[0:1, 1:2], op=AOP.max)
            nc.vector.reciprocal(out=sml[0:1, 1:2], in_=sml[0:1, 0:1])
            nc.vector.tensor_tensor(out=sml[0:1, 2:3], in0=psum_s[0:1, 1:2],
                                    in1=sml[0:1, 1:2], op=AOP.mult)

            # broadcast recip down partitions; att = masked * recip
            psum_rb = psml.tile([128, 1], FP, tag="ps")
            nc.tensor.matmul(psum_rb, lhsT=ones_sb[0:1, :], rhs=sml[0:1, 1:2],
                             start=True, stop=True)
            rb_sb = work.tile([128, 1], FP)
            nc.scalar.activation(out=rb_sb, in_=psum_rb, func=AFT.Copy)
            att = work.tile([128, NT], FP)
            nc.vector.tensor_scalar(out=att, in0=masked, scalar1=rb_sb,
                                    scalar2=None, op0=AOP.mult)

            # attention out: transpose [128,16] -> [16,128], DMA to DRAM
            psum_attT = psml.tile([16, 128], FP, tag="ps")
            nc.tensor.transpose(psum_attT, att, ident_sb)
            attT_sb = work.tile([16, 128], FP)
            nc.scalar.activation(out=attT_sb, in_=psum_attT, func=AFT.Copy)
            nc.gpsimd.dma_start(out=att_out[b].rearrange("(i t) -> i t", i=NT),
                                in_=attT_sb)

            # ctxh[1, H] = sum_t att[t] * listener[b, t, :]
            psum_ctxh = pctxh.tile([1, H], FP, tag="pctxh")
            for i in range(NT):
                nc.tensor.matmul(psum_ctxh, lhsT=att[:, i:i + 1],
                                 rhs=lts[i // TG][:, i % TG, :],
                                 start=(i == 0), stop=(i == NT - 1))
            ctxh_sb = work.tile([1, H], FP)
            nc.scalar.activation(out=ctxh_sb, in_=psum_ctxh, func=AFT.Copy)

            # transpose ctxh [1,512] -> [128,4] (4 column transposes, one bank)
            psum_ctxhT = psml.tile([128, 4], FP, tag="ps")
            for j in range(4):
                nc.tensor.matmul(psum_ctxhT[:, j:j + 1],
                                 lhsT=ctxh_sb[0:1, 128 * j:128 * (j + 1)],
                                 rhs=ident_sb[0:1, 0:1], is_transpose=True,
                                 start=(j == 0), stop=(j == 3))
            ctxhT_sb = work.tile([128, 4], FP)
            nc.scalar.activation(out=ctxhT_sb, in_=psum_ctxhT, func=AFT.Copy)

            # context[1, V] = ctxh @ WvT + bv*asum
            psum_ctx = psml.tile([1, V], FP, tag="ps")
            for j in range(4):
                nc.tensor.matmul(psum_ctx, lhsT=ctxhT_sb[:, j:j + 1],
                                 rhs=wvT_sb[:, j, :], start=(j == 0), stop=(j == 3))
            bva = work.tile([1, V], FP)
            nc.vector.tensor_scalar(out=bva, in0=bv_sb, scalar1=sml[0:1, 2:3],
                                    scalar2=None, op0=AOP.mult)
            ctx_sb = work.tile([1, V], FP)
            nc.vector.tensor_tensor(out=ctx_sb, in0=psum_ctx, in1=bva, op=AOP.add)
            nc.gpsimd.dma_start(out=ctx_out[b:b + 1, :], in_=ctx_sb)

    nc.compile()
    return nc


def get_nc():
    if "nc" not in _CACHE:
        _CACHE["nc"] = _build()
    return _CACHE["nc"]


def make_in_maps(decoder_state, listener_output, outputs_length, Ws, bs, Wh, Wv, bv):
    iota_np = (np.arange(128)[:, None] + 128 * np.arange(NT)[None, :]).astype(np.float32)
    ident_np = np.eye(128, dtype=np.float32)
    ones_np = np.ones((128, 128), dtype=np.float32)
    wsT_np = np.ascontiguousarray(Ws.T, dtype=np.float32)
    wh_np = np.ascontiguousarray(Wh, dtype=np.float32)
    wvT_np = np.ascontiguousarray(Wv.T, dtype=np.float32)
    bs_np = np.ascontiguousarray(np.asarray(bs, dtype=np.float32).reshape(K, 1))
    bv_np = np.ascontiguousarray(np.asarray(bv, dtype=np.float32).reshape(1, V))
    in_maps = []
    for core in range(N_CORES):
        sl = slice(B_LOC * core, B_LOC * (core + 1))
        lens_c = np.asarray(outputs_length[sl], dtype=np.float32).copy()
        if core == 0:
            lens_c[0] = float(T)  # reference unmasks global sample 0
        in_maps.append({
            "lst": np.ascontiguousarray(listener_output[sl], dtype=np.float32),
            "decT": np.ascontiguousarray(np.asarray(decoder_state[sl]).T,
                                         dtype=np.float32),
            "lens": lens_c.reshape(1, B_LOC),
            "wsT": wsT_np,
            "wh": wh_np,
            "wvT": wvT_np,
            "bs2": bs_np,
            "bv2": bv_np,
            "iota": iota_np,
            "ident": ident_np,
            "ones": ones_np,
        })
    return in_maps


def kernel(decoder_state, listener_output, outputs_length, Ws, bs, Wh, bh, Wv, bv):
    from concourse.bass_utils import run_bass_kernel_spmd

    decoder_state = np.asarray(decoder_state, dtype=np.float32)
    listener_output = np.asarray(listener_output, dtype=np.float32)
    outputs_length = np.asarray(outputs_length)
    nc = get_nc()
    in_maps = make_in_maps(decoder_state, listener_output, outputs_length,
                           Ws, bs, Wh, Wv, bv)
    res = run_bass_kernel_spmd(nc, in_maps, core_ids=list(range(N_CORES)))
    context = np.concatenate([res.results[c]["ctx_out"] for c in range(N_CORES)], 0)
    att = np.concatenate([res.results[c]["att_out"] for c in range(N_CORES)], 0)
    return context.astype(np.float32), att.reshape(B, 1, T).astype(np.float32)
